# revision 60
# baseline (speedup 1.0000x reference)
"""Trainium2 Bass kernel for nn_CMR_59931973648949 (gnn_message_passing).

Contract: kernel(**inputs) takes FULL unsharded numpy inputs and returns the
FULL [16, 1024] output. Data-parallel over batch across 8 cores (2 samples
per core, weights replicated). Weights are host-packed partition-major
([128, F]); W_v/W_node/W_rel-fused weights ship as e4m3 (per-tensor scales
folded into the q/u0/u1 psum drains), W_out as fp16 in two column-halves.

Math per sample (refactored):
  scl[n] = mean(norm_w)/max(||visf[:,n]||,1e-12)   (applied on feat_v drain)
  feat_v = visf.T @ W_v.T * scl ; used via its transpose ftT2
  q/u0/u1 from node/relate reps with WnT=W_node.T/sqrt(DV),
      WA0/1=W_rel.T@W_e[:, :DV | DV:]/sqrt(DE)  (e4m3, dequant on drain;
      A0/A1 additionally halved for the tanh form of sigmoid)
  find = softmax(mask(q @ feat_vT)) * node_mask
  ea_r = tanh((A0[r,:] bcast + A1T[:,r])/2)  [sigmoid = 0.5 tanh + 0.5,
      the affine is folded into the h-stage drain via gs[r]]
  g_findT = find.T-gather via GT (folds valid*relate_mask*onehot(obj))
  h[r,:] = g_find[r,:] @ ea_r ; find2T = findT + h.T @ ST (onehot(subj))
  fa = rowmax(find2T); fa /= max(max(fa),1); fa = fa*bm + (1-bm)*1e-7
  mem = visf @ fa ; out = mem @ W_out.T + b_out

Both samples are batched on the partition axis everywhere (rows 0-11/32-43
for 12-row tensors via tile_position col 32, rows 0-63/64-127 for 64-row
tensors via col 64); diagonal blocks of I128 provide identities at matching
partition bases. The two mem reductions run on DVE and Pool concurrently.
"""

import numpy as np

import concourse.bass as bass
import concourse.tile as tile
from concourse import bacc, mybir
from concourse.bass_utils import run_bass_kernel_spmd

P = 128
B, K, R, N = 16, 12, 12, 64
DW, DV, DVIS, DE, DC = 512, 512, 2048, 512, 1024
NCORES = 8
S = B // NCORES  # samples per core = 2
N2 = S * N  # 128: both samples' boxes side by side
K2 = S * K  # 24
K44 = 32 + K  # batched 12-row tensors: s0 rows 0..12, s1 rows 32..44

F32 = mybir.dt.float32
F16 = mybir.dt.float16
F8E4 = mybir.dt.float8e4
HALF = F16
CBLK = DC // P  # 8 column blocks of 128 output channels

# rest (fp16) column layout: shared masks then per-sample block.
# bm masks are [44, 128]: they also mask out the wrong-sample column half
# of the batched [44, 128] logits (rows 0-11 live in cols 0-63, rows 32-43
# in cols 64-127), so the batched softmax reduces correctly over 128 cols.
_SH_BA = 0            # bm2add [44, 128] (additive softmax mask)
_SH_NM = 128          # nm2col [44, 1]
_SH_GS = 129          # gs2 [128, 12] (0.5 * sum_k nm[k] G[r,k], tanh affine,
#                       pre-broadcast down partitions; s0 rows 0-63, s1 64-127)
_SH_RM = 141          # rmask2 [128, 64]
_SH_RT = 205          # t1/t0 ratio (fp16, replicated) [128, 1]
SHARED_F = 206
_SM_GT = 0            # GT [12@32s, 12]
_SM_ST = 12           # ST [12@32s, 12]
_SM_FM = 24           # famul [1, 64]
_SM_FA = 88           # faadd [1, 64]
SMALLS_F = 152
REST_F = SHARED_F + S * SMALLS_F

# dequant scales are folded into the host-packed reps (x 2**12 to stay in
# fp16 range); drains unscale by the compile-time 2**-12. u1 additionally
# multiplies by the shipped t1/t0 ratio. oscl carries only the bias and is
# DMA'd only when the bias is nonzero.
QSH = float(2.0 ** -12)
NSCL = CBLK

KP = K44  # reps padded to the 44-col batched layout (junk cols are zero)
WCAT_F = P + 2 * 4 * KP  # I128 | nrepT2 | rrepT2

_cache = {}


def _pack(a):
    """[(o*128), F] row-major -> [128, o*F] partition-major."""
    o = a.shape[0] // P
    return np.ascontiguousarray(
        a.reshape(o, P, a.shape[1]).transpose(1, 0, 2).reshape(P, -1)
    )


def build_nc(bm_ones=False, nm_ones=False, rm_ones=False, bz=False):
    nc = bacc.Bacc(num_devices=NCORES)

    # visf | I128 | reps | rest in ONE fp16 stream (fewer DMA overheads)
    d_wcat = nc.declare_dram_parameter(
        "wcat16", [P, 16 * N2 + WCAT_F + REST_F], HALF, isOutput=False
    )
    d_WvT = nc.declare_dram_parameter("WvT8", [P, 16 * DV], F8E4, isOutput=False)
    d_wq8 = nc.declare_dram_parameter("wq8", [P, 3 * 4 * DV], F8E4, isOutput=False)
    d_oscl = nc.declare_dram_parameter("oscl", [P, NSCL], F32, isOutput=False)
    d_WoT = nc.declare_dram_parameter("WoT16", [P, 16 * DC], HALF, isOutput=False)
    # output is channel-major [DC, S]; the host transposes (free)
    d_out = nc.declare_dram_parameter("out", [DC, S], F16, isOutput=True)

    TANH = rm_ones  # tanh form needs ea scale folded into h; rmask breaks it

    with tile.TileContext(nc) as tc:
        with (
            tc.tile_pool(name="singles", bufs=1) as singles,
            tc.tile_pool(name="ps", bufs=2) as ps,
            tc.tile_pool(name="psum", bufs=8, space="PSUM") as psum,
        ):
            # ---- DMA stream (SP queue runs in order) ----
            VW = 16 * N2
            wcat_sb = singles.tile([P, VW + WCAT_F + REST_F], HALF)
            nc.sync.dma_start(out=wcat_sb[:, :VW], in_=d_wcat[:, :VW])
            visf2_mm = wcat_sb[:, :VW].rearrange("p (o n) -> p o n", o=16)
            I128_sb = wcat_sb[:, VW : VW + P]
            reps_sb = wcat_sb[:, VW + P : VW + WCAT_F].rearrange(
                "p (t o k) -> p t o k", t=2, o=4
            )
            # [P, 4, 44] each; cols 0-11 sample 0, 32-43 sample 1, rest zero
            nrep2 = reps_sb[:, 0]
            rrep2 = reps_sb[:, 1]
            WvT4 = singles.tile([P, 4, 4, DV], F8E4)
            nc.sync.dma_start(
                out=WvT4[:, :2],
                in_=d_WvT[:, : 8 * DV].rearrange("p (a o d) -> p a o d", a=2, o=4),
            )
            nc.sync.dma_start(out=wcat_sb[:, VW:], in_=d_wcat[:, VW:])
            nc.sync.dma_start(
                out=WvT4[:, 2:],
                in_=d_WvT[:, 8 * DV :].rearrange("p (a o d) -> p a o d", a=2, o=4),
            )
            WvT_sb = WvT4[:].rearrange("p a o d -> p (a o) d")
            rest_sb = wcat_sb[:, VW + WCAT_F :]
            if not bz:
                oscl_sb = singles.tile([P, NSCL], F32)
                nc.sync.dma_start(out=oscl_sb[:], in_=d_oscl[:])
            wq8_sb = singles.tile([P, 3, 4, DV], F8E4)
            nc.sync.dma_start(
                out=wq8_sb[:], in_=d_wq8[:].rearrange("p (t o d) -> p t o d", t=3, o=4)
            )
            WoT_sb = singles.tile([P, 16, DC], HALF)
            WOT_PIECES = ((0, CBLK),)
            nc.sync.dma_start(
                out=WoT_sb[:],
                in_=d_WoT[:].rearrange("p (o c) -> p o c", o=16),
            )

            ones_1xP = singles.tile([1, P], HALF)
            nc.vector.memset(ones_1xP[:], 1.0)
            warm_ps = psum.tile([1, P], F32, tag="ps", name="warm")
            for _ in range(33):
                nc.tensor.matmul(
                    out=warm_ps[:], lhsT=ones_1xP[:, :1], rhs=ones_1xP[:],
                    start=True, stop=True,
                )

            bm2add = rest_sb[:K44, _SH_BA : _SH_BA + N2]
            nm2col = rest_sb[:K44, _SH_NM : _SH_NM + 1]
            gs2 = rest_sb[:, _SH_GS : _SH_GS + R]
            rmask2 = rest_sb[:, _SH_RM : _SH_RM + N]
            sm = [rest_sb[:, SHARED_F + s * SMALLS_F :] for s in range(S)]

            def GTm(s):
                return sm[s][32 * s : 32 * s + K, _SM_GT : _SM_GT + R]

            def STm(s):
                return sm[s][32 * s : 32 * s + R, _SM_ST : _SM_ST + K]

            # ---- gram -> column norms -> scl (1/||.||, s_mean folded in Wv) ----
            gram_ps = psum.tile([N2, N2], F32, tag="ps")
            for c in range(16):
                nc.tensor.matmul(
                    out=gram_ps[:],
                    lhsT=visf2_mm[:, c, :],
                    rhs=visf2_mm[:, c, :],
                    start=(c == 0),
                    stop=(c == 15),
                )

            # ---- feat_v for both samples [n2, 512] (drain applies scl) ----
            featv_ps = psum.tile([N2, DV], F32, tag="ps")
            for c in range(16):
                nc.tensor.matmul(
                    out=featv_ps[:],
                    lhsT=visf2_mm[:, c, :],
                    rhs=WvT_sb[:, c, :],
                    start=(c == 0),
                    stop=(c == 15),
                )

            # scl chain (DVE+Act) — emitted before lin_T so the in-order DVE
            # queue reaches it as soon as the gram psum stops
            gd_sb = ps.tile([N2, N2], F32, tag="gd")
            nc.vector.tensor_tensor(
                out=gd_sb[:], in0=gram_ps[:], in1=I128_sb[:],
                op=mybir.AluOpType.mult,
            )
            scl = singles.tile([N2, 1], F32)
            nc.vector.tensor_reduce(
                out=scl[:], in_=gd_sb[:], axis=mybir.AxisListType.X,
                op=mybir.AluOpType.add,
            )
            nc.scalar.sqrt(out=scl[:], in_=scl[:])
            # dummy exp pulls the exp/tanh act-table load into Act idle time
            # (the auto-inserted load otherwise lands on the critical path)
            dume = singles.tile([1, 1], F32)
            nc.scalar.activation(
                out=dume[:], in_=scl[:1, :1],
                func=mybir.ActivationFunctionType.Exp,
            )
            nc.vector.tensor_scalar_max(out=scl[:], in0=scl[:], scalar1=1e-12)
            nc.vector.reciprocal(out=scl[:], in_=scl[:])
            featv_sb = singles.tile([N2, DV], HALF)
            nc.vector.tensor_scalar_mul(out=featv_sb[:], in0=featv_ps[:], scalar1=scl[:])

            # ---- qT/u0T/u1T for both samples [d, 44], e4m3 weights ----
            # dequant scales are pre-folded into the reps; drains unscale by
            # the constant 2**-12 (u1 also by the shipped t1/t0 ratio)
            ratio32 = singles.tile([P, 1], F32)
            nc.vector.tensor_copy(
                out=ratio32[:], in_=rest_sb[:, _SH_RT : _SH_RT + 1]
            )

            def lin_T(w_sb, x_ap, name, ratio=None):
                out_ps = psum.tile([P, 4, KP], F32, tag="ps", name=name + "_ps")
                for dc in range(4):
                    for wc in range(4):
                        nc.tensor.matmul(
                            out=out_ps[:, dc, :],
                            lhsT=w_sb[:, wc, P * dc : P * (dc + 1)],
                            rhs=x_ap[:, wc, :],
                            start=(dc == 0 and wc == 0),
                            stop=(dc == 3 and wc == 3),
                        )
                out_sb = singles.tile([P, 4, KP], HALF, name=name)
                if ratio is None:
                    nc.vector.tensor_scalar_mul(
                        out=out_sb[:], in0=out_ps[:], scalar1=QSH
                    )
                else:
                    nc.vector.scalar_tensor_tensor(
                        out=out_sb[:], in0=out_ps[:], scalar=QSH,
                        in1=ratio[:, None, :].to_broadcast([P, 4, KP]),
                        op0=mybir.AluOpType.mult, op1=mybir.AluOpType.mult,
                    )
                return out_sb

            qT2_sb = lin_T(wq8_sb[:, 0], nrep2, "qT2")

            ftT2_ps = psum.tile([P, 4, N2], F32, tag="ps")
            for c in range(4):
                nc.tensor.matmul(
                    out=ftT2_ps[:, c, :],
                    lhsT=featv_sb[:, P * c : P * (c + 1)],
                    rhs=I128_sb[:],
                    start=(c == 0),
                    stop=(c == 3),
                )
            ftT2_sb = singles.tile([P, 4, N2], HALF)
            nc.vector.tensor_copy(out=ftT2_sb[:], in_=ftT2_ps[:])

            u0T2_sb = lin_T(wq8_sb[:, 1], rrep2, "u0T2")
            u1T2_sb = lin_T(wq8_sb[:, 2], rrep2, "u1T2", ratio=ratio32)

            # ---- batched attention: logits / A0 / A1 [44, 64] ----
            def att_mm(qsb, name):
                # [44, 128] in one region: junk lhsT cols are zero, each
                # sample's valid block is rows 32s..32s+12 x cols 64s..64s+64
                out_ps = psum.tile([K44, N2], F32, tag="ps", name=name)
                for c in range(4):
                    nc.tensor.matmul(
                        out=out_ps[:],
                        lhsT=qsb[:, c, :],
                        rhs=ftT2_sb[:, c, :],
                        start=(c == 0), stop=(c == 3),
                    )
                return out_ps

            lg_ps = att_mm(qT2_sb, "lg_ps")
            # additive mask only (logits are << 6e4, so add beats mul+add);
            # it also blanks the wrong-sample column half of the batched tile
            lg_sb = ps.tile([K44, N2], F32, name="lg", tag="lg")
            nc.vector.tensor_tensor(
                out=lg_sb[:], in0=lg_ps[:], in1=bm2add, op=mybir.AluOpType.add
            )
            nmx = ps.tile([K44, 1], F32, tag="nmx")
            nc.vector.tensor_reduce(
                out=nmx[:], in_=lg_sb[:], axis=mybir.AxisListType.X,
                op=mybir.AluOpType.max, negate=True,
            )
            e_sb = ps.tile([K44, N2], F32, tag="e")
            ssum = ps.tile([K44, 1], F32, tag="ss")
            nc.scalar.activation(
                out=e_sb[:], in_=lg_sb[:],
                func=mybir.ActivationFunctionType.Exp,
                bias=nmx[:], scale=1.0, accum_out=ssum[:],
            )
            rs = ps.tile([K44, 1], F32, tag="rs")
            nc.vector.reciprocal(out=rs[:], in_=ssum[:])
            if not nm_ones:
                nc.vector.tensor_tensor(
                    out=rs[:], in0=rs[:], in1=nm2col, op=mybir.AluOpType.mult
                )
            find_sb = ps.tile([K44, N2], HALF, tag="find")
            nc.vector.tensor_scalar_mul(out=find_sb[:], in0=e_sb[:], scalar1=rs[:])

            A0_ps = att_mm(u0T2_sb, "A0_ps")
            A0_sb = ps.tile([K44, N2], HALF, tag="A0")
            nc.scalar.copy(out=A0_sb[:], in_=A0_ps[:])
            A1_ps = att_mm(u1T2_sb, "A1_ps")
            A1_sb = ps.tile([K44, N2], HALF, tag="A1")
            nc.scalar.copy(out=A1_sb[:], in_=A1_ps[:])

            # ---- gfT2 [128, R] and find2T init [128, K] (both samples) ----
            gfT2_ps = psum.tile([N2, R], F32, tag="ps", name="gfT2")
            for s in range(S):
                nc.tensor.matmul(
                    out=gfT2_ps[64 * s : 64 * s + N, :],
                    lhsT=find_sb[32 * s : 32 * s + K, N * s : N * (s + 1)],
                    rhs=GTm(s),
                    start=True, stop=True,
                    tile_position=(32 * s, 64 * s),
                    skip_group_check=True,
                )
            gfT2_sb = ps.tile([N2, R], HALF, tag="gfT2sb")
            nc.scalar.copy(out=gfT2_sb[:], in_=gfT2_ps[:])

            f2T2_ps = psum.tile([N2, K], F32, tag="ps", name="f2T2")
            for s in range(S):
                nc.tensor.matmul(
                    out=f2T2_ps[64 * s : 64 * s + N, :],
                    lhsT=find_sb[32 * s : 32 * s + K, N * s : N * (s + 1)],
                    rhs=I128_sb[32 * s : 32 * s + K, 32 * s : 32 * s + K],
                    start=True, stop=True,
                    tile_position=(32 * s, 64 * s),
                    skip_group_check=True,
                )
            # drained early (off the critical path): the fused add+rowmax may
            # read only one PSUM operand on hardware
            f2Ta_sb = ps.tile([N2, K], HALF, tag="f2Ta")
            nc.scalar.copy(out=f2Ta_sb[:], in_=f2T2_ps[:])

            # ---- edge attention ea [128, 12, 64]: tanh((A0+A1)/2) ----
            act_fn = (
                mybir.ActivationFunctionType.Tanh
                if TANH else mybir.ActivationFunctionType.Sigmoid
            )
            ea_all = ps.tile([N2, R, N], HALF, tag="ea")
            GR = R // 2
            for g in range(2):
                Bg = psum.tile([N2, GR, N], F32, tag="ps", name=f"B6_{g}")
                for s in range(S):
                    for i in range(GR):
                        r = GR * g + i
                        sel = I128_sb[
                            32 * s : 32 * s + K, 32 * s + r : 32 * s + r + 1
                        ].to_broadcast([K, N])
                        nc.tensor.matmul(
                            out=Bg[64 * s : 64 * s + N, i, :],
                            lhsT=sel,
                            rhs=A0_sb[32 * s : 32 * s + K, N * s : N * (s + 1)],
                            start=(i == 0), stop=False,
                            tile_position=(32 * s, 64 * s),
                            skip_group_check=True,
                        )
                        nc.tensor.matmul(
                            out=Bg[64 * s : 64 * s + N, i, :],
                            lhsT=A1_sb[32 * s : 32 * s + K, N * s : N * (s + 1)],
                            rhs=sel,
                            start=False, stop=(i == GR - 1),
                            tile_position=(32 * s, 64 * s),
                            skip_group_check=True,
                        )
                nc.scalar.activation(
                    out=ea_all[:, GR * g : GR * (g + 1), :], in_=Bg[:], func=act_fn
                )
            if not rm_ones:
                nc.vector.tensor_tensor(
                    out=ea_all[:],
                    in0=ea_all[:],
                    in1=rmask2[:, None, :].to_broadcast([N2, R, N]),
                    op=mybir.AluOpType.mult,
                )

            # ---- h-stage: hT2 [128, R] batched; transpose per sample ----
            hT2_ps = psum.tile([N2, R], F32, tag="ps", name="hT2")
            for g in range(2):
                for s in range(S):
                    for i in range(GR):
                        r = GR * g + i
                        nc.tensor.matmul(
                            out=hT2_ps[64 * s : 64 * s + N, r : r + 1],
                            lhsT=ea_all[64 * s : 64 * s + N, r, :],
                            rhs=gfT2_sb[64 * s : 64 * s + N, r : r + 1],
                            start=(r == 0), stop=(r == R - 1),
                            tile_position=(64 * s, 64 * s),
                            skip_group_check=True,
                        )
            hT2_sb = ps.tile([N2, R], HALF, tag="hT2sb")
            if TANH:
                # sigmoid = 0.5*tanh + 0.5: h = 0.5*h_tanh + gs (gs pre-halved
                # and pre-broadcast down partitions host-side)
                nc.vector.scalar_tensor_tensor(
                    out=hT2_sb[:], in0=hT2_ps[:], scalar=0.5, in1=gs2,
                    op0=mybir.AluOpType.mult, op1=mybir.AluOpType.add,
                )
            else:
                nc.vector.tensor_copy(out=hT2_sb[:], in_=hT2_ps[:])

            h2_ps = psum.tile([K44, N], F32, tag="ps", name="h2")
            for s in range(S):
                nc.tensor.matmul(
                    out=h2_ps[32 * s : 32 * s + R, :],
                    lhsT=hT2_sb[64 * s : 64 * s + N, :],
                    rhs=I128_sb[64 * s : 64 * s + N, 64 * s : 64 * s + N],
                    start=True, stop=True,
                    tile_position=(64 * s, 32 * s),
                    skip_group_check=True,
                )
            h2_sb = ps.tile([K44, N], HALF, tag="h2sb")
            nc.vector.tensor_copy(out=h2_sb[:R, :], in_=h2_ps[:R, :])
            nc.scalar.copy(out=h2_sb[32 : 32 + R, :], in_=h2_ps[32 : 32 + R, :])
            # scatter contribution in its own psum; summed on the drain (a
            # far-apart split accumulation group trips the scheduler)
            f2Tb_ps = psum.tile([N2, K], F32, tag="ps", name="f2Tb")
            for s in range(S):
                nc.tensor.matmul(
                    out=f2Tb_ps[64 * s : 64 * s + N, :],
                    lhsT=h2_sb[32 * s : 32 * s + R, :],
                    rhs=STm(s),
                    start=True, stop=True,
                    tile_position=(32 * s, 64 * s),
                    skip_group_check=True,
                )

            # ---- final attention + mem ----
            # fused: f2sum = f2Ta + f2Tb, fa2 = rowmax(f2sum) in one DVE pass
            f2sum = ps.tile([N2, K], HALF, tag="f2sum")
            fa2_sb = ps.tile([N2, 1], HALF, tag="fa2")
            with nc.allow_low_precision("fa rowmax in fp16"):
                nc.vector.tensor_tensor_reduce(
                    out=f2sum[:], in0=f2Ta_sb[:], in1=f2Tb_ps[:],
                    scale=1.0, scalar=0.0,
                    op0=mybir.AluOpType.add, op1=mybir.AluOpType.max,
                    accum_out=fa2_sb[:],
                )
            faT2_ps = psum.tile([1, N2], F32, tag="ps", name="faT2")
            nc.tensor.matmul(
                out=faT2_ps[:], lhsT=fa2_sb[:], rhs=I128_sb[:],
                start=True, stop=True,
            )
            faT2_sb = ps.tile([1, N2], HALF, tag="faT2sb")
            nr2 = ps.tile([1, S, 1], F32, tag="nr2")
            nc.vector.tensor_reduce(
                out=nr2[:],
                in_=faT2_ps[:].rearrange("o (s n) -> o s n", s=S),
                axis=mybir.AxisListType.X, op=mybir.AluOpType.max,
            )
            nc.vector.tensor_scalar_max(out=nr2[:], in0=nr2[:], scalar1=1.0)
            nc.vector.tensor_tensor(
                out=faT2_sb[:].rearrange("o (s n) -> o s n", s=S),
                in0=faT2_ps[:].rearrange("o (s n) -> o s n", s=S),
                in1=nr2[:].to_broadcast([1, S, N]),
                op=mybir.AluOpType.divide,
            )
            if not bm_ones:
                for s in range(S):
                    nc.vector.tensor_tensor(
                        out=faT2_sb[:, N * s : N * (s + 1)],
                        in0=faT2_sb[:, N * s : N * (s + 1)],
                        in1=sm[s][:1, _SM_FM : _SM_FM + N],
                        op=mybir.AluOpType.mult,
                    )
                    nc.vector.tensor_tensor(
                        out=faT2_sb[:, N * s : N * (s + 1)],
                        in0=faT2_sb[:, N * s : N * (s + 1)],
                        in1=sm[s][:1, _SM_FA : _SM_FA + N],
                        op=mybir.AluOpType.add,
                    )
            fabc2_ps = psum.tile([P, N2], F32, tag="ps", name="fabc2")
            nc.tensor.matmul(
                out=fabc2_ps[:], lhsT=ones_1xP[:], rhs=faT2_sb[:],
                start=True, stop=True,
            )
            fabc2_sb = ps.tile([P, N2], HALF, tag="fabc2sb")
            nc.scalar.copy(out=fabc2_sb[:], in_=fabc2_ps[:])

            # Pool does the broadcast multiplies, DVE the free-axis reduces
            # (Pool cannot reduce along X); reduce s0 overlaps multiply s1
            mem2r_sb = singles.tile([P, 16, S], HALF)
            wtmp = [None, None]
            for s in range(S):
                ns = slice(N * s, N * (s + 1))
                wtmp[s] = ps.tile([P, 16, N], HALF, tag=f"wtmp{s}", name=f"wtmp{s}")
                src_ap = fabc2_ps if s == 0 else fabc2_sb
                (nc.vector if s == 0 else nc.gpsimd).tensor_tensor(
                    out=wtmp[s][:],
                    in0=visf2_mm[:, :, ns],
                    in1=src_ap[:, None, ns].to_broadcast([P, 16, N]),
                    op=mybir.AluOpType.mult,
                )
            for s in range(S):
                with nc.allow_low_precision("fp16 mem rounding, matches cast"):
                    nc.vector.tensor_reduce(
                        out=mem2r_sb[:, :, s], in_=wtmp[s][:],
                        axis=mybir.AxisListType.X, op=mybir.AluOpType.add,
                    )

            # ---- W_out: weights stationary, 2-wide moving rhs. The tail
            # (matmul/drain/DMA) runs per weight piece; the output leaves in
            # channel-major layout via a strided DMA (no transposes). ----
            for pi, (cb0, ncb) in enumerate(WOT_PIECES):
                o_ps = psum.tile([P, ncb, S], F32, tag="ps", name=f"o_ps{pi}")
                for cbl in range(ncb):
                    for c in range(16):
                        nc.tensor.matmul(
                            out=o_ps[:, cbl, :],
                            lhsT=WoT_sb[:, c, P * (cb0 + cbl) : P * (cb0 + cbl + 1)],
                            rhs=mem2r_sb[:, c, :],
                            start=(cbl == 0 and c == 0),
                            stop=(cbl == ncb - 1 and c == 15),
                            skip_group_check=True,
                        )
                outT_sb = singles.tile([P, ncb, S], HALF, name=f"outT{pi}")
                nc.scalar.copy(out=outT_sb[:], in_=o_ps[:])
                if not bz:
                    nc.vector.tensor_tensor(
                        out=outT_sb[:], in0=outT_sb[:],
                        in1=oscl_sb[:, cb0 : cb0 + ncb, None].to_broadcast(
                            [P, ncb, S]
                        ),
                        op=mybir.AluOpType.add,
                    )
                nc.sync.dma_start(
                    out=d_out[P * cb0 : P * (cb0 + ncb), :].rearrange(
                        "(c p) s -> p c s", p=P
                    ),
                    in_=outT_sb[:],
                )

    nc.finalize()
    return nc


def _host_prep(inputs):
    node_rep = np.asarray(inputs["node_rep"], np.float32)
    relate_rep = np.asarray(inputs["relate_rep"], np.float32)
    relate_os = np.asarray(inputs["relate_os"])
    relate_mask = np.asarray(inputs["relate_mask"], np.float32)
    vision_feat = np.asarray(inputs["vision_feat"], np.float32)
    relation_mask = np.asarray(inputs["relation_mask"], np.float32)
    box_mask = np.asarray(inputs["box_mask"], np.float32)
    node_mask = np.asarray(inputs["node_mask"], np.float32)
    norm_w = np.asarray(inputs["norm_w"], np.float32)
    W_v = np.asarray(inputs["W_v"], np.float32)
    W_e = np.asarray(inputs["W_e"], np.float32)
    W_node = np.asarray(inputs["W_node"], np.float32)
    W_rel = np.asarray(inputs["W_rel"], np.float32)
    W_out = np.asarray(inputs["W_out"], np.float32)
    b_out = np.asarray(inputs["b_out"], np.float32)

    s_mean = np.float32(np.mean(norm_w))
    WvT = (W_v.T * s_mean).astype(np.float32)
    WnT = (W_node.T / np.float32(np.sqrt(DV))).astype(np.float32)
    WA0 = (W_rel.T @ W_e[:, :DV] / np.float32(np.sqrt(DE))).astype(np.float32)
    WA1 = (W_rel.T @ W_e[:, DV:] / np.float32(np.sqrt(DE))).astype(np.float32)
    WoT = np.ascontiguousarray(W_out.T)

    import ml_dtypes

    E4 = ml_dtypes.float8_e4m3
    fmax8 = np.float32(ml_dtypes.finfo(E4).max)

    def q8(a):
        s = np.float32(max(np.max(np.abs(a)), 1e-30) / fmax8)
        return (a / s).astype(E4), s

    Wv8, sv = q8(WvT)
    Wn8, tn = q8(WnT)
    WA08, t0 = q8(WA0)
    WA18, t1 = q8(WA1)

    rm_ones = bool(np.all(relation_mask == 1.0))
    tanh_form = rm_ones

    subj = relate_os[..., 1].astype(np.int64)
    obj = relate_os[..., 0].astype(np.int64)
    valid = (subj != -1).astype(np.float32)
    obj_c = np.clip(obj, 0, K - 1)
    subj_c = np.clip(subj, 0, K - 1)
    G = np.zeros((B, R, K), np.float32)
    STm = np.zeros((B, R, K), np.float32)
    bi = np.arange(B)[:, None]
    ri = np.arange(R)[None, :]
    G[bi, ri, obj_c] = valid * relate_mask
    STm[bi, ri, subj_c] = 1.0
    # tanh affine term: gs[b, r] = 0.5 * sum_k node_mask[b, k] * G[b, r, k]
    gs = 0.5 * np.einsum("bk,brk->br", node_mask, G).astype(np.float32)

    bmmul = (box_mask > 0).astype(np.float32)
    bmadd = (bmmul - 1.0) * np.float32(6e4)  # fp16-safe large negative
    famul = box_mask
    faadd = (1.0 - box_mask) * np.float32(1e-7)

    WvT_p = _pack(Wv8.astype(np.float32)).astype(E4)
    wq8_p = np.concatenate(
        [
            _pack(Wn8.astype(np.float32)),
            _pack(WA08.astype(np.float32)),
            _pack(WA18.astype(np.float32)),
        ],
        axis=1,
    ).astype(E4)
    WoT16_p = _pack(WoT).astype(np.float16)
    I128 = np.eye(P, dtype=np.float32)

    oscl = np.ascontiguousarray(b_out.reshape(CBLK, P).T)
    half_f = np.float32(0.5) if tanh_form else np.float32(1.0)
    qn_fold = np.float32(tn * sv / QSH)
    q0_fold = np.float32(t0 * sv * half_f / QSH)
    ratio10 = np.float32(t1 / t0)

    in_maps = []
    for core in range(NCORES):
        b0 = S * core
        visf2 = np.concatenate(
            [_pack(vision_feat[b]).reshape(P, 16, N) for b in range(b0, b0 + S)],
            axis=2,
        ).reshape(P, -1)
        # reps in the padded 44-col layout (cols 32s..32s+12 per sample),
        # with the dequant scales folded in (x 2**12 against fp16 underflow)
        nrep2 = np.zeros((P, 4, KP), np.float32)
        rrep2 = np.zeros((P, 4, KP), np.float32)
        for s in range(S):
            b = b0 + s
            nrep2[:, :, 32 * s : 32 * s + K] = qn_fold * _pack(
                np.ascontiguousarray(node_rep[b].T)
            ).reshape(P, 4, K)
            rrep2[:, :, 32 * s : 32 * s + R] = q0_fold * _pack(
                np.ascontiguousarray(relate_rep[b].T)
            ).reshape(P, 4, R)
        wcat_head = np.concatenate(
            [I128, nrep2.reshape(P, -1), rrep2.reshape(P, -1)], axis=1
        )

        rest = np.zeros((P, REST_F), np.float32)
        rest[:, _SH_RT] = ratio10
        # default: everything masked (add -6e4) including junk rows
        rest[:K44, _SH_BA : _SH_BA + N2] = np.float32(-6e4)
        for s in range(S):
            b = b0 + s
            r0 = 32 * s
            c0n = N * s
            rest[r0 : r0 + K, _SH_BA + c0n : _SH_BA + c0n + N] = bmadd[b][None, :]
            rest[r0 : r0 + K, _SH_NM] = node_mask[b]
            rest[64 * s : 64 * s + N, _SH_RM : _SH_RM + N] = relation_mask[b]
            rest[64 * s : 64 * s + N, _SH_GS : _SH_GS + R] = gs[b][None, :]
            c0 = SHARED_F + s * SMALLS_F
            rest[r0 : r0 + K, c0 + _SM_GT : c0 + _SM_GT + R] = G[b].T
            rest[r0 : r0 + R, c0 + _SM_ST : c0 + _SM_ST + K] = STm[b]
            rest[0, c0 + _SM_FM : c0 + _SM_FM + N] = famul[b]
            rest[0, c0 + _SM_FA : c0 + _SM_FA + N] = faadd[b]

        m = {
            "wcat16": np.ascontiguousarray(
                np.concatenate([visf2, wcat_head, rest], axis=1)
            ).astype(np.float16),
            "WvT8": WvT_p,
            "wq8": wq8_p,
            "oscl": oscl,
            "WoT16": WoT16_p,
        }
        in_maps.append(m)
    return in_maps


def kernel(**inputs) -> np.ndarray:
    bm_ones = bool(np.all(np.asarray(inputs["box_mask"]) == 1.0))
    nm_ones = bool(np.all(np.asarray(inputs["node_mask"]) == 1.0))
    rm_ones = bool(np.all(np.asarray(inputs["relation_mask"]) == 1.0))
    bz = bool(np.all(np.asarray(inputs["b_out"]) == 0.0))
    key = ("nc", bm_ones, nm_ones, rm_ones, bz)
    if key not in _cache:
        _cache[key] = build_nc(bm_ones, nm_ones, rm_ones, bz)
    nc = _cache[key]
    in_maps = _host_prep(inputs)
    res = run_bass_kernel_spmd(nc, in_maps, core_ids=list(range(NCORES)))
    outs = [np.asarray(res.results[c]["out"], np.float32).T for c in range(NCORES)]
    return np.concatenate(outs, axis=0)
